# revision 8
# baseline (speedup 1.0000x reference)
"""Gabor transform Trainium2 kernel.

result[b, f, t] = sum_l signal[b, l] * window[t, l] * cos(2*pi*f*l/L)

B=32, L=8192, T=F=128. Data-parallel over batch across 8 cores (4 batches
per core). The window/cos dictionaries are input-independent and are
precomputed on the host (bf16, replicated to every core).

Per-core device program, chunking l into 64 chunks of 128:
  - rhs[l, b*128+t] = window_T[l, t] * signal[b, l]   (DVE/ACT, per-partition
    scalar multiply, split 2:1 across the two engines)
  - psum[f, b*128+t] += cos_T[l_chunk, f].T @ rhs     (PE, bf16, N=512)
accumulated over all 64 chunks into a single PSUM bank, then copied out.

Sync-layout constraint: the trn2 Activation descriptor only fits ONE sync
wait, so every ACT/DVE instruction must depend on at most one semaphore.
Achieved by (a) shipping each window piece and the signal columns it needs
in a single DMA (one sem), (b) rhs pool bufs=NCHUNK so no slot-reuse WAR
waits, (c) single-engine PSUM->SBUF copy.
"""

import numpy as np
import ml_dtypes

import concourse.bass as bass
import concourse.tile as tile
from concourse import mybir
from concourse.bass_utils import run_bass_kernel_spmd

B, L, T, F = 32, 8192, 128, 128
SIGMA = 0.1
NCORES = 8
BLOC = B // NCORES          # 4 batches per core
NCHUNK = L // 128           # 64 l-chunks
NPIECE = 4                  # dictionary DMA pieces (1 ct + NPIECE ws + 1 out
                            # DMA must stay <= 8 so no HWDGE queue is reused,
                            # which would add a second sync wait to the out DMA)
CPP = NCHUNK // NPIECE      # chunks per piece
NFREE = BLOC * T            # 512 moving columns
WCOLS = CPP * T             # 1024 window cols per piece
SCOLS = CPP * BLOC          # 32 signal cols per piece
PCOLS = WCOLS + SCOLS       # 1056 cols per ws piece

_bf16 = ml_dtypes.bfloat16


def _make_dicts():
    """Host-side window/cos dictionaries in bf16.

    wt_re[lp, chunk*128 + t] = window[t, chunk*128 + lp]
    ct_re[lp, chunk*128 + f] = cos_mod[f, chunk*128 + lp]
    """
    tc_ = np.linspace(0.0, L - 1, T, dtype=np.float32).astype(np.int32).astype(np.float32)
    fv = np.linspace(0.0, L // 2, F, dtype=np.float32).astype(np.int32).astype(np.float32)
    t = np.arange(L, dtype=np.float32)
    sigma_w = np.float32(SIGMA * L)
    w = np.exp(-0.5 * ((t[None, :] - tc_[:, None]) / sigma_w) ** 2).astype(np.float32)  # [T, L]
    two_pi = np.float32(2.0 * np.pi)
    cm = np.cos(two_pi * fv[:, None] * t[None, :] / np.float32(L)).astype(np.float32)   # [F, L]
    wt_re = w.T.reshape(NCHUNK, 128, T).transpose(1, 0, 2).reshape(128, NCHUNK * T)
    ct_re = cm.T.reshape(NCHUNK, 128, F).transpose(1, 0, 2).reshape(128, NCHUNK * F)
    return wt_re.astype(_bf16), ct_re.astype(_bf16)


def _build():
    nc = bass.Bass()
    ws_d = nc.declare_dram_parameter("ws", [128, NPIECE * PCOLS], mybir.dt.bfloat16, isOutput=False)
    ct_d = nc.declare_dram_parameter("ct", [128, NCHUNK * F], mybir.dt.bfloat16, isOutput=False)
    out_d = nc.declare_dram_parameter("out", [128, NFREE], mybir.dt.float32, isOutput=True)

    with tile.TileContext(nc) as tc:
        with (
            tc.tile_pool(name="dicts", bufs=1) as dict_pool,
            tc.tile_pool(name="rhs", bufs=NCHUNK) as rhs_pool,
            tc.tile_pool(name="psum", bufs=1, space=bass.MemorySpace.PSUM) as psum_pool,
            tc.tile_pool(name="outp", bufs=1) as out_pool,
        ):
            # all of cos_T in one DMA -> one DMAHW sem for everything PE reads
            ct_t = dict_pool.tile([128, NCHUNK * F], mybir.dt.bfloat16, tag="ct")
            nc.sync.dma_start(ct_t[:], ct_d[:])

            ws_tiles, sig_dve, sig_act = [], [], []
            for p in range(NPIECE):
                ws_t = dict_pool.tile([128, PCOLS], mybir.dt.bfloat16, tag=f"ws{p}")
                nc.sync.dma_start(ws_t[:], ws_d[:, p * PCOLS:(p + 1) * PCOLS])
                ws_tiles.append(ws_t)
                # fp32 signal columns, one private copy per consuming engine so
                # windowing ops only ever depend on same-engine data + 1 DMA sem
                sd = dict_pool.tile([128, SCOLS], mybir.dt.float32, tag=f"sd{p}")
                nc.vector.tensor_copy(sd[:], ws_t[:, WCOLS:PCOLS])
                sig_dve.append(sd)
                sa = dict_pool.tile([128, SCOLS], mybir.dt.float32, tag=f"sa{p}")
                nc.scalar.copy(sa[:], ws_t[:, WCOLS:PCOLS])
                sig_act.append(sa)

            psum = psum_pool.tile([128, NFREE], mybir.dt.float32)
            # Dummy matmul whose only purpose is to absorb the ct-DMA wait
            # into PE program order (the MM descriptor fits a single sync
            # wait, so real matmuls can only wait on their rhs producer).
            scratch = psum_pool.tile([128, NFREE], mybir.dt.float32, tag="scratch")
            nc.tensor.matmul(scratch[:], ct_t[:, :F], ct_t[:, :NFREE], start=True, stop=True)

            for i in range(NCHUNK):
                p, j = divmod(i, CPP)
                ws_t = ws_tiles[p]
                wt_ap = ws_t[:, j * T:(j + 1) * T]
                rhs = rhs_pool.tile([128, NFREE], mybir.dt.bfloat16)
                on_dve = (i % 3) != 2  # 2:1 DVE:ACT, whole chunks
                for b in range(BLOC):
                    col = j * BLOC + b
                    dst = rhs[:, b * T:(b + 1) * T]
                    if on_dve:
                        nc.vector.tensor_scalar_mul(dst, wt_ap, sig_dve[p][:, col:col + 1])
                    else:
                        nc.scalar.mul(dst, wt_ap, sig_act[p][:, col:col + 1])
                nc.tensor.matmul(
                    psum[:],
                    ct_t[:, i * F:(i + 1) * F],
                    rhs[:],
                    start=(i == 0),
                    stop=(i == NCHUNK - 1),
                )

            out_s = out_pool.tile([128, NFREE], mybir.dt.float32)
            nc.vector.tensor_copy(out_s[:], psum[:])
            nc.sync.dma_start(out_d[:], out_s[:])
    _legalize_drain(nc)
    return nc


def _legalize_drain(nc):
    """This walrus build accepts a single sync wait per instruction. The
    kernel body is built so every instruction naturally has <=1 wait, but
    Tile's final Drain waits on every proc. In this kernel all of those are
    transitively implied by out-DMA completion (out DMA waits DVE copy ->
    DVE waited PE -> PE waited ACT / input DMAs), so keep only the out-DMA
    queue wait on the drain."""
    import bass_rust

    drain = None
    waited_elsewhere = set()
    for f in nc.m.functions:
        for bb in f.blocks:
            for inst in bb.instructions:
                si = inst.sync_info
                if si is None:
                    continue
                if inst.opcode == "Drain" and len(si.on_wait) > 1:
                    drain = inst
                else:
                    for w in si.on_wait:
                        waited_elsewhere.add(w.ant_name)
    assert drain is not None
    keep = [w for w in drain.sync_info.on_wait if w.ant_name not in waited_elsewhere]
    assert len(keep) == 1, [w.ant_name for w in drain.sync_info.on_wait]
    drain.sync_info = bass_rust.SyncInfo(on_wait=keep, on_update=list(drain.sync_info.on_update))
    # sanity: nothing else carries >1 wait
    for f in nc.m.functions:
        for bb in f.blocks:
            for inst in bb.instructions:
                si = inst.sync_info
                if si is not None and len(si.on_wait) > 1:
                    raise RuntimeError(f"{inst.name} {inst.opcode} has {len(si.on_wait)} waits")


_CACHE = {}


def _get_nc():
    if "nc" not in _CACHE:
        _CACHE["nc"] = _build()
        _CACHE["dicts"] = _make_dicts()
    return _CACHE["nc"], _CACHE["dicts"]


def _in_maps(signal):
    _, (wt_re, ct_re) = _get_nc()
    maps = []
    for c in range(NCORES):
        s = np.ascontiguousarray(signal[c * BLOC:(c + 1) * BLOC]).astype(np.float32)
        # sig_re[lp, chunk*4 + b] = signal[b, chunk*128 + lp]
        sig_re = s.reshape(BLOC, NCHUNK, 128).transpose(2, 1, 0).reshape(128, NCHUNK * BLOC)
        sig_bf = sig_re.astype(_bf16)
        # interleave: per piece p -> [wt cols (1024) | sig cols (32)]
        ws = np.empty((128, NPIECE * PCOLS), dtype=_bf16)
        for p in range(NPIECE):
            ws[:, p * PCOLS: p * PCOLS + WCOLS] = wt_re[:, p * WCOLS:(p + 1) * WCOLS]
            ws[:, p * PCOLS + WCOLS:(p + 1) * PCOLS] = sig_bf[:, p * SCOLS:(p + 1) * SCOLS]
        maps.append({"ws": np.ascontiguousarray(ws), "ct": ct_re})
    return maps


def _unshard(results):
    out = np.empty((B, F, T), dtype=np.float32)
    for c in range(NCORES):
        r = results[c]["out"]  # [128, 512], col = b*128 + t
        out[c * BLOC:(c + 1) * BLOC] = r.reshape(F, BLOC, T).transpose(1, 0, 2)
    return out


def kernel(signal: np.ndarray) -> np.ndarray:
    nc, _ = _get_nc()
    res = run_bass_kernel_spmd(nc, _in_maps(signal), list(range(NCORES)))
    return _unshard(res.results)


def kernel_traced(signal: np.ndarray, **trace_kwargs):
    """Like kernel() but with tracing, for test.py."""
    nc, _ = _get_nc()
    res = run_bass_kernel_spmd(
        nc, _in_maps(signal), list(range(NCORES)), trace=True, **trace_kwargs
    )
    return _unshard(res.results), res
